# revision 15
# baseline (speedup 1.0000x reference)
"""CrissCrossAttention (full HW-token attention) Trainium2 kernel.

Reference computation (B=4, C=256, H=W=64, N=H*W=4096, CQK=32):
    q = wq@x+bq   [B,32,N]
    k = wk@x+bk   [B,32,N]
    v = wv@x+bv   [B,256,N]
    energy = q^T k      [B,N,N]
    attn = softmax_j(energy)
    out = v @ attn^T    [B,256,N]
    final = x + wg@out + bg

Sharding: 8 cores = 4 batches x 2 query-halves. Each core receives x[b]
rolled so its 2048 query columns are always columns 0:2048 (softmax over
keys is permutation invariant, so rolling keys+values consistently leaves
the result unchanged) -> one identical SPMD program for all cores.

Algebraic folding done on host:
    wg@(v@attn^T)+bg = (wg@wv)@x@attn^T + (wg@bv+bg)    (attn rows sum to 1)
so the kernel only needs W=wg@wv and b=wg@bv+bg, and the g-projection
matmul disappears.

The k bias is dropped entirely: softmax_j((q_i+bq).(k_j+bk)) =
softmax_j((q_i+bq).k_j) because the bk terms are constant in j and cancel.

Device layout trick: energy tiles are computed transposed, S_t[j,i]=k^T q,
so the exp'd tile P_t[j,i] feeds the AV matmul directly as the stationary
operand (no transposes anywhere in the main loop). A ones-column appended
to vW_t produces the softmax denominator inside the same accumulation.
All matmul operands are float32r (FP22-truncated reads, full PE rate).

The energy matmul contracts over only CQK=32 of 128 PE rows; k and q are
replicated 4x across partition groups so the 2 key-subtiles of a j-batch
run in distinct 32-row PE tile positions, which the PE executes
concurrently. HW constraint (found by bisection): a small-K/row-tiled
matmul must write PSUM at a 2KB-bank-aligned offset, which caps a 2-bank
energy tile at 2 subtiles of 512 queries.

Structure: a projection pre-phase paced by the streamed x chunks (PSUM
staging borrows the attention-loop rings), then the attention loop.  Per
i-tile the 16 j-batches are processed as TWO interleaved key-half chains
(keys 0:2048 and 2048:4096) accumulating into the same PSUM tiles - the
two exp->AV dependency chains alternate on ACT/PE so the exp+semaphore
latency of one chain hides under the other chain's matmuls.
"""

import sys

import numpy as np

_B, _C, _H, _W = 4, 256, 64, 64
_N = _H * _W  # 4096 key/value positions
_CQK = _C // 8  # 32
_NCORES = 8
_NQ = _N // 2  # 2048 queries per core

# Filled by kernel() for the benefit of test harnesses; never read here.
LAST_RUN_INFO = {}
TRACE = False

_REPO = "/opt/trn_rl_repo"


def _ensure_path():
    if _REPO not in sys.path:
        sys.path.insert(0, _REPO)


def build_program(n=_N, nq=_NQ, reps=1):
    """Build the single-core Bass/Tile program (identical across cores).

    n:    number of key/value positions    (multiple of 1024)
    nq:   number of query positions        (multiple of 512)
    reps: repeat the compute body in a HW loop (benchmarking only)
    """
    _ensure_path()
    import concourse.tile as tile
    from concourse import bacc, mybir
    from concourse.bass import ds, ts

    f32 = mybir.dt.float32
    f32r = mybir.dt.float32r
    bf16 = mybir.dt.bfloat16
    Exp = mybir.ActivationFunctionType.Exp
    Copy = mybir.ActivationFunctionType.Copy
    mult = mybir.AluOpType.mult
    add = mybir.AluOpType.add

    P = 128
    IW = 512  # query-tile width for the energy matmul
    jpb = 2  # key j-subtiles (128 keys each) batched per PSUM/exp tile
    AVW = 257  # AV matmul moving width: 256 vW cols + 1 ones col
    XCH = 512  # x streams in 512-column chunks
    assert n % (2 * jpb * P) == 0 and nq % IW == 0
    NJ = n // 128  # j-tiles of 128 keys
    NJB = NJ // jpb  # j batches
    NJH = NJB // 2  # j batches per key-half chain
    NI = nq // IW  # i-tiles of IW queries
    NSL = IW // P  # i-slices per i-tile

    nc = bacc.Bacc("TRN2", target_bir_lowering=False, debug=False)

    x_in = nc.dram_tensor("x_in", [_C, n], f32r, kind="ExternalInput")
    xqt_b = nc.dram_tensor("xqt_b", [nq, _C], f32, kind="ExternalInput")
    wq4t = nc.dram_tensor("wq4t", [_C, 128], f32r, kind="ExternalInput")
    wk4t = nc.dram_tensor("wk4t", [_C, 128], f32r, kind="ExternalInput")
    bq4 = nc.dram_tensor("bq4", [128, 1], f32, kind="ExternalInput")
    Wt = nc.dram_tensor("Wt", [_C, _C], f32r, kind="ExternalInput")
    out_t = nc.dram_tensor("out_t", [nq, _C], f32, kind="ExternalOutput")

    with tile.TileContext(nc) as tc:
        with (
            tc.tile_pool(name="singles", bufs=1) as singles,
            tc.tile_pool(name="ptile", bufs=4) as ppool,
            tc.tile_pool(name="epi", bufs=4) as epool,
            tc.tile_pool(name="spsum", bufs=2, space="PSUM") as spool,
            tc.tile_pool(name="accpsum", bufs=4, space="PSUM") as accpool,
        ):
            # ---- persistent SBUF tensors ----
            x_sb = [
                singles.tile([P, n], f32r, tag=f"x{c}", name=f"x_sb{c}")
                for c in range(2)
            ]
            k4_sb = singles.tile([P, n], f32r, tag="k4")
            q4_sb = singles.tile([P, nq], f32r, tag="q4")
            vW1_sb = singles.tile([P, NJ, AVW], bf16, tag="vw1")
            wq4_sb = [
                singles.tile([P, 128], f32r, tag=f"wq{c}", name=f"wq4_sb{c}")
                for c in range(2)
            ]
            wk4_sb = [
                singles.tile([P, 128], f32r, tag=f"wk{c}", name=f"wk4_sb{c}")
                for c in range(2)
            ]
            Wt_sb = [
                singles.tile([P, _C], f32r, tag=f"wt{c}", name=f"Wt_sb{c}")
                for c in range(2)
            ]
            bq4_sb = singles.tile([P, 1], f32, tag="bq")
            # residual (x^T + b) staged once; epilogue reads slices
            xqt_sb = singles.tile([P, nq // P, _C], f32, tag="xqt")

            # ones column -> softmax denominator rides along the AV matmul
            nc.vector.memset(vW1_sb[:, :, 256:AVW], 1.0)

            # SP-queue DMA order is priority order: q/k weights, first x
            # chunk, remaining weights, the x stream, then the bulky
            # residual tensor (needed ~30us in, lands by ~20us).
            for c in range(2):
                nc.sync.dma_start(out=wk4_sb[c], in_=wk4t[c * P : (c + 1) * P, :])
                nc.sync.dma_start(out=wq4_sb[c], in_=wq4t[c * P : (c + 1) * P, :])
            nc.sync.dma_start(out=bq4_sb, in_=bq4[:, :])
            for c in range(2):
                nc.sync.dma_start(
                    out=x_sb[c][:, 0:XCH], in_=x_in[c * P : (c + 1) * P, 0:XCH]
                )
            for c in range(2):
                nc.sync.dma_start(out=Wt_sb[c], in_=Wt[c * P : (c + 1) * P, :])
            for t in range(1, n // XCH):
                for c in range(2):
                    nc.sync.dma_start(
                        out=x_sb[c][:, ts(t, XCH)],
                        in_=x_in[c * P : (c + 1) * P, ts(t, XCH)],
                    )
            nc.sync.dma_start(
                out=xqt_sb, in_=xqt_b[:, :].rearrange("(t p) c -> p t c", p=P)
            )

            def compute_body():
                # ---- projection pre-phase, paced by the x chunk stream ----
                for ch in range(n // XCH):
                    # k for key chunk ch (4x replicated); no bias needed
                    kp = accpool.tile([P, XCH], f32, tag="acc", name="kp")
                    for c in range(2):
                        nc.tensor.matmul(
                            kp,
                            wk4_sb[c][:, :],
                            x_sb[c][:, ts(ch, XCH)],
                            start=(c == 0),
                            stop=(c == 1),
                        )
                    nc.vector.tensor_copy(k4_sb[:, ts(ch, XCH)], kp)

                    # vW_t[j, c] = (W @ x)^T = x^T @ W^T; the 4 j-tiles of
                    # the chunk in one 2-bank PSUM tile (the sp ring, idle
                    # until the attention loop) -> one wide ACT evacuation
                    vp = spool.tile([P, 4, _C], f32, tag="s", name="vp")
                    for u in range(4):
                        j = ch * 4 + u
                        for c in range(2):
                            nc.tensor.matmul(
                                vp[:, u, :],
                                x_sb[c][:, ts(j, P)],
                                Wt_sb[c][:, :],
                                start=(c == 0),
                                stop=(c == 1),
                            )
                    nc.scalar.activation(vW1_sb[:, ds(ch * 4, 4), 0:256], vp, Copy)

                    # q (+bq): the query columns span the first NI chunks
                    if ch < NI:
                        qp = accpool.tile([P, IW], f32, tag="acc", name="qp")
                        for c in range(2):
                            nc.tensor.matmul(
                                qp,
                                wq4_sb[c][:, :],
                                x_sb[c][:, ts(ch, IW)],
                                start=(c == 0),
                                stop=(c == 1),
                            )
                        nc.vector.tensor_scalar_add(
                            q4_sb[:, ts(ch, IW)], qp, bq4_sb[:, :]
                        )

                # ---- attention main loop: two interleaved key-half chains ----
                def emit_energy(i, jb):
                    # S_t[j, i] = sum_d k[d, j] * q[d, i]  (K = 32); the two
                    # j-subtiles run concurrently in PE row groups 0 and 32
                    # (k/q are replicated across partition groups for this),
                    # each writing its own bank-aligned PSUM half.
                    sp = spool.tile([P, jpb * IW], f32, tag="s", name="sp")
                    for t in range(jpb):
                        jt = jb * jpb + t
                        nc.tensor.matmul(
                            sp[:, ts(t, IW)],
                            k4_sb[32 * t : 32 * t + _CQK, ts(jt, P)],
                            q4_sb[32 * t : 32 * t + _CQK, ts(i, IW)],
                            start=True,
                            stop=True,
                            tile_position=(32 * t, 0),
                        )
                    return sp

                def emit_exp(sp):
                    pt = ppool.tile([P, jpb * IW], bf16, tag="p", name="pt")
                    nc.scalar.activation(pt, sp, Exp)
                    return pt

                def emit_av(accs, pt, jb, start, stop):
                    for t in range(jpb):
                        jt = jb * jpb + t
                        for s in range(NSL):
                            nc.tensor.matmul(
                                accs[s],
                                pt[:, ds(t * IW + s * P, P)],
                                vW1_sb[:, jt, :],
                                start=(start and t == 0),
                                stop=(stop and t == jpb - 1),
                            )

                def emit_epilogue(accs, i):
                    # out = acc * (1/denom) + (x^T + b)
                    for s in range(NSL):
                        isl = i * IW + s * P
                        rc = epool.tile([P, 1], f32, tag="rc", name="rc")
                        nc.vector.reciprocal(rc, accs[s][:, 256:257])
                        st = epool.tile([P, _C], f32, tag="st", name="st")
                        nc.vector.scalar_tensor_tensor(
                            st,
                            accs[s][:, 0:256],
                            rc[:, :],
                            xqt_sb[:, i * NSL + s, :],
                            op0=mult,
                            op1=add,
                        )
                        nc.sync.dma_start(out=out_t[isl : isl + P, :], in_=st)

                # chain A covers j-batches 0:NJH, chain B covers NJH:NJB;
                # both accumulate into the same acc tiles (adds commute,
                # emission order fixes which MM carries start/stop)
                sp_a = emit_energy(0, 0)
                sp_b = emit_energy(0, NJH)
                for i in range(NI):
                    accs = [
                        accpool.tile([P, AVW], f32, tag="acc", name="acc")
                        for _ in range(NSL)
                    ]
                    for m in range(NJH):
                        pt_a = emit_exp(sp_a)
                        pt_b = emit_exp(sp_b)
                        emit_av(accs, pt_a, m, start=(m == 0), stop=False)
                        if m + 1 < NJH:
                            sp_a = emit_energy(i, m + 1)
                        elif i + 1 < NI:
                            sp_a = emit_energy(i + 1, 0)
                        emit_av(accs, pt_b, NJH + m, start=False, stop=(m == NJH - 1))
                        if m + 1 < NJH:
                            sp_b = emit_energy(i, NJH + m + 1)
                        elif i + 1 < NI:
                            sp_b = emit_energy(i + 1, NJH)
                    emit_epilogue(accs, i)

            if reps > 1:
                with tc.For_i(0, reps, 1, hint_engines=(mybir.EngineType.PE,)):
                    compute_body()
            else:
                compute_body()

    nc.compile()
    return nc


def _host_inputs(x, wq, bq, wk, bk, wv, bv, wg, bg, n=_N, nq=_NQ):
    """Per-core input maps (numpy only)."""
    xf = np.ascontiguousarray(x.reshape(_B, _C, n).astype(np.float32))
    W64 = wg.astype(np.float64) @ wv.astype(np.float64)
    b64 = wg.astype(np.float64) @ bv.astype(np.float64) + bg.astype(np.float64)
    Wt = np.ascontiguousarray(W64.T.astype(np.float32))
    bcomb = b64.astype(np.float32)
    wq4t = np.ascontiguousarray(np.tile(wq.T.astype(np.float32), (1, 4)))
    wk4t = np.ascontiguousarray(np.tile(wk.T.astype(np.float32), (1, 4)))
    bq4 = np.ascontiguousarray(np.tile(bq.astype(np.float32), 4)[:, None])

    halves = n // nq
    in_maps = []
    for core in range(_NCORES):
        b, half = core // halves, core % halves
        off = half * nq
        x_roll = np.ascontiguousarray(np.roll(xf[b], -off, axis=1))
        xqt_b = np.ascontiguousarray(x_roll[:, :nq].T + bcomb[None, :])
        in_maps.append(
            {
                "x_in": x_roll,
                "xqt_b": xqt_b,
                "wq4t": wq4t,
                "wk4t": wk4t,
                "bq4": bq4,
                "Wt": Wt,
            }
        )
    return in_maps


def kernel(x, wq, bq, wk, bk, wv, bv, wg, bg):
    _ensure_path()
    from concourse.bass_utils import run_bass_kernel_spmd

    nc = build_program()
    in_maps = _host_inputs(x, wq, bq, wk, bk, wv, bv, wg, bg)
    core_ids = list(range(_NCORES))
    res = run_bass_kernel_spmd(nc, in_maps, core_ids, trace=TRACE)
    LAST_RUN_INFO["exec_time_ns"] = res.exec_time_ns
    LAST_RUN_INFO["mean_exec_time_ns"] = res.mean_exec_time_ns
    LAST_RUN_INFO["results"] = res

    out = np.empty((_B, _C, _N), np.float32)
    for core in range(_NCORES):
        b, off = core // 2, (core % 2) * _NQ
        out[b, :, off : off + _NQ] = res.results[core]["out_t"].T
    return out.reshape(_B, _C, _H, _W)
